# revision 43
# baseline (speedup 1.0000x reference)
"""Data-parallel 3x3 conv2d (stride 1, pad 1) on 8 Trainium2 NeuronCores.

Problem: x [32, 64, 112, 112] f32, weight [128, 64, 3, 3] f32, bias [128]
-> out [32, 128, 112, 112] f32.

Sharding: batch N=32 split 4 images per core across 8 cores; weight/bias
replicated (forward only, no collectives needed).

Design (bf16 implicit GEMM, 5 matmul slabs per output tile):
  - Host pre-pads and pre-shifts the input into a DRAM tensor T0
    [128, 13110] bf16 per image: partitions 0-63 ("A") hold the 64
    channels of xpad (114x114) flattened, partitions 64-127 ("B") hold
    the same shifted up one padded row (B[o] = xpad[o+114]).  One
    contiguous DMA per chunk loads it; no on-device scatter is needed.
  - The DVE builds a second tile T1 from T0 with two column-shifted
    copies: A1[o] = T0A[o+228] (= xpad row+2), B1[o] = T0B[o+115]
    (= xpad row+2, col+1).  This gives a halves-pair with a 1-column
    delta, so the two kh=2 taps (kw=0,1) pack into ONE dense matmul.
  - Each PSUM tile covers 4 output rows (454 moving columns incl. the
    interior pad columns) and accumulates 5 bf16 K=128 matmuls:
      s0..s2: T0 @ col offsets 0,1,2   -> taps (kh0,kw) + (kh1,kw)
      s3:     T1 @ col offset 0        -> taps (kh2,kw0) + (kh2,kw1)
      s4:     T0 @ col offset 116      -> tap  (kh2,kw2) (A-half weights 0)
    (vs 6 fp32r matmuls in the f32 baseline -- 17% less PE time, and bf16
    gets the compiler's fast-weight-load so LDWEIGHTS hides behind
    streaming: measured ~193ns per 454-col matmul back-to-back, zero
    PE idle over the whole stream.)
  - Epilogue: ScalarE activation(Identity, bias) PSUM->SBUF bf16,
    dropping pad columns; batched contiguous DMAs store bf16 to DRAM;
    the host converts back to f32.  bf16 in/out keeps total HBM traffic
    at ~26 MB/core vs ~51 MB for the f32 double-load baseline.
  - Numerics: bf16 x/w quantization + bf16 output rounding gives rel
    err ~2.9e-3 (f32 PSUM accumulation), well under the 2e-2 gate.
  Queues: input loads on SP(sync) HWDGE, stores on ScalarE HWDGE.
"""
import sys

if '/opt/trn_rl_repo' not in sys.path:
    sys.path.insert(0, '/opt/trn_rl_repo')

import numpy as np

N, CIN, HH, WW = 32, 64, 112, 112
OC = 128
NCORES = 8
N_PER_CORE = N // NCORES

WP = HH + 2              # 114 padded row length
FLAT = WP * WP           # 12996
LFLAT = FLAT + WP        # 13110: B half is shifted one row, needs the tail
LT1 = 12768              # T1 columns actually read (27*456 + 456)
RPT = 4                  # output rows per PSUM tile
NCOL = RPT * WP          # 456 moving columns per matmul
NT = HH // RPT           # 28 tiles per image
SH_A1 = 2 * WP           # T1 A-half column shift (row+2)
SH_B1 = WP + 1           # T1 B-half column shift rel. to T0 B (row+2,col+1)

_cache = {}


def _build():
    import concourse.bacc as bacc
    import concourse.mybir as mybir
    from concourse.tile import TileContext

    F32 = mybir.dt.float32
    BF16 = mybir.dt.bfloat16

    nc = bacc.Bacc("TRN2", target_bir_lowering=False, debug=False,
                   num_devices=NCORES)
    xp = nc.declare_dram_parameter("xp", [N_PER_CORE, 128, LFLAT], BF16,
                                   isOutput=False)
    wt = nc.declare_dram_parameter("wt", [128, 5 * 128], BF16, isOutput=False)
    bias = nc.declare_dram_parameter("bias", [128, 1], F32, isOutput=False)
    y = nc.declare_dram_parameter("y", [N_PER_CORE, OC, HH * WW], BF16,
                                  isOutput=True)
    xa = xp.ap()
    ya = y.ap()

    # T0 load chunks and T1 build chunks (column ranges).  Image 0 uses a
    # finer split so the first tiles' data is ready ASAP after the NEFF
    # preamble; later images load while the previous one computes.
    B0 = [0, 2622, 5244, 7866, 10488, LFLAT]
    B1 = [0, 2622, 5244, 7866, 10488, LT1]
    B0F = [0, 1539, 3078, 5700, 8322, 10944, LFLAT]
    B1F = [0, 1311, 2622, 5244, 7866, 10488, LT1]

    with TileContext(nc) as tc:
        with (
            tc.tile_pool(name="wpool", bufs=1) as wpool,
            tc.tile_pool(name="xpool", bufs=1) as xpool,
            tc.tile_pool(name="opool", bufs=4) as opool,
            tc.tile_pool(name="pspool", bufs=8, space="PSUM") as pspool,
        ):
            # wt/bias ride the ScalarE HWDGE so the SP queue's first trigger
            # is image-0 chunk 0.  (Measured: the prologue floor is ~11us on
            # every queue layout tried -- the first SP-queue DMA bytes can't
            # flow before ~8.6us and tile-0 data+weights land ~11.3us.)
            wtile = wpool.tile([128, 5 * 128], BF16, tag="w")
            nc.scalar.dma_start(out=wtile[:, :], in_=wt[:, :])
            btile = wpool.tile([128, 1], F32, tag="b")
            nc.scalar.dma_start(out=btile[:, :], in_=bias[:, :])
            # memset scratch: warm-up matmul operand (values are irrelevant
            # -- results are discarded), so warm-ups have no DMA dependency
            # and the PE p-state ramp burns during the preamble
            wutile = wpool.tile([128, 384], BF16, tag="wu")
            nc.gpsimd.memset(wutile[:, :], 0.0)

            t0s = [xpool.tile([128, LFLAT], BF16, tag=f"t0{i}",
                              name=f"t0{i}") for i in range(2)]
            t1s = [xpool.tile([128, LT1], BF16, tag=f"t1{i}",
                              name=f"t1{i}") for i in range(2)]

            def load_image(n, b0=B0, b1=B1):
                t0, t1 = t0s[n % 2], t1s[n % 2]
                for q in range(len(b0) - 1):
                    a, b = b0[q], b0[q + 1]
                    nc.sync.dma_start(out=t0[:, a:b], in_=xa[n, :, a:b])
                for q in range(len(b1) - 1):
                    a, b = b1[q], b1[q + 1]
                    nc.vector.tensor_copy(t1[0:64, a:b],
                                          t0[0:64, a + SH_A1:b + SH_A1])
                    nc.vector.tensor_copy(t1[64:128, a:b],
                                          t0[64:128, a + SH_B1:b + SH_B1])

            NMM = NCOL - 2   # 454: the last row's 2 pad columns are never read

            def compute_image(n, groups, store_eng=None):
                store_eng = store_eng or nc.scalar
                t0, t1 = t0s[n % 2], t1s[n % 2]
                ot = None
                t = 0
                # NOTE: a tile's 5 accumulating matmuls MUST stay back-to-back
                # on one PSUM bank -- interleaving two tiles' chains across
                # banks measured +73ns/matmul (149.5us total); same-bank
                # consecutive accumulation is the PE fast path.
                for g in groups:
                    ot = opool.tile([128, 4 * RPT * WW], BF16, tag="o")
                    for gi in range(g):
                        f0 = t * NCOL
                        ps = pspool.tile([128, NCOL], F32, tag="ps")
                        for s in range(3):
                            nc.tensor.matmul(
                                ps[:, 0:NMM], wtile[:, s * 128:(s + 1) * 128],
                                t0[:, f0 + s: f0 + s + NMM],
                                start=(s == 0), stop=False)
                        nc.tensor.matmul(
                            ps[:, 0:NMM], wtile[:, 3 * 128:4 * 128],
                            t1[:, f0: f0 + NMM], start=False, stop=False)
                        nc.tensor.matmul(
                            ps[:, 0:NMM], wtile[:, 4 * 128:5 * 128],
                            t0[:, f0 + WP + 2: f0 + WP + 2 + NMM],
                            start=False, stop=True)
                        half = gi * RPT * WW
                        psv = ps[:, :].rearrange("o (r t) -> o r t",
                                                 r=RPT, t=WP)[:, :, 0:WW]
                        otv = ot[:, half:half + RPT * WW].rearrange(
                            "o (r t) -> o r t", r=RPT, t=WW)
                        nc.scalar.activation(
                            otv, psv, mybir.ActivationFunctionType.Identity,
                            bias=btile[:, :])
                        t += 1
                    store_eng.dma_start(
                        out=ya[n, :, (t - g) * RPT * WW: t * RPT * WW],
                        in_=ot[:, 0:g * RPT * WW])

            # dep-free warm-up matmuls (on never-written scratch) run during
            # the NEFF preamble/first loads, so the PE HAM clock-gate reaches
            # 8/8 before the first real matmul
            for _ in range(14):
                psw = pspool.tile([128, 256], F32, tag="ps", name="psw")
                nc.tensor.matmul(psw[:, :], wutile[:, 0:128],
                                 wutile[:, 128:384], start=True, stop=True)

            load_image(0, b0=B0F, b1=B1F)
            for n in range(N_PER_CORE):
                if n + 1 < N_PER_CORE:
                    load_image(n + 1)
                    compute_image(n, groups=[4] * 7)
                else:
                    # last image: finer store batching, final store a single
                    # tile, and stores on the (idle) SP queue so the HWDGE
                    # config overlaps the final ScalarE drain instead of
                    # serializing behind it
                    compute_image(n, groups=[2] * 13 + [1, 1],
                                  store_eng=nc.sync)
    nc.compile()
    return nc


def _pack_weights(weight: np.ndarray):
    """[O=128, C=64, 3, 3] -> [128, 5*128] bf16 slab layout (k-major).

    Slabs 0-2: [w(kh0,kw) | w(kh1,kw)]; slab 3: [w(kh2,0) | w(kh2,1)];
    slab 4: [0 | w(kh2,2)]."""
    from ml_dtypes import bfloat16
    w5 = np.zeros((5, 128, 128), np.float32)   # [slab, k, o]
    wf = weight.astype(np.float32)
    for kw in range(3):
        w5[kw, 0:64] = wf[:, :, 0, kw].T
        w5[kw, 64:128] = wf[:, :, 1, kw].T
    w5[3, 0:64] = wf[:, :, 2, 0].T
    w5[3, 64:128] = wf[:, :, 2, 1].T
    w5[4, 64:128] = wf[:, :, 2, 2].T
    return np.ascontiguousarray(
        w5.transpose(1, 0, 2).reshape(128, 5 * 128)).astype(bfloat16)


def _pack_x(xc: np.ndarray):
    """[4, 64, 112, 112] f32 -> padded, row-shifted [4, 128, LFLAT] bf16."""
    from ml_dtypes import bfloat16
    xq = xc.astype(bfloat16)
    xpad = np.zeros((N_PER_CORE, CIN, WP, WP), bfloat16)
    xpad[:, :, 1:1 + HH, 1:1 + WW] = xq
    flat = xpad.reshape(N_PER_CORE, CIN, FLAT)
    t0 = np.zeros((N_PER_CORE, 128, LFLAT), bfloat16)
    t0[:, 0:64, 0:FLAT] = flat
    t0[:, 64:128, 0:FLAT - WP] = flat[:, :, WP:]
    return t0


def kernel(x: np.ndarray, weight: np.ndarray, bias: np.ndarray,
           _trace: bool = False) -> np.ndarray:
    from concourse.bass_utils import run_bass_kernel_spmd

    x = np.ascontiguousarray(np.asarray(x, dtype=np.float32))
    weight = np.asarray(weight, dtype=np.float32)
    bias = np.asarray(bias, dtype=np.float32)
    assert x.shape == (N, CIN, HH, WW), x.shape
    assert weight.shape == (OC, CIN, 3, 3), weight.shape
    assert bias.shape == (OC,), bias.shape

    if 'nc' not in _cache:
        _cache['nc'] = _build()
    nc = _cache['nc']

    wtp = _pack_weights(weight)
    bp = np.ascontiguousarray(bias.reshape(128, 1))
    in_maps = [
        {"xp": _pack_x(x[N_PER_CORE * i: N_PER_CORE * (i + 1)]),
         "wt": wtp, "bias": bp}
        for i in range(NCORES)
    ]
    res = run_bass_kernel_spmd(nc, in_maps, core_ids=list(range(NCORES)),
                               trace=_trace)
    out = np.concatenate([
        np.asarray(res.results[i]["y"]).astype(np.float32) for i in
        range(NCORES)], axis=0).reshape(N, OC, HH, WW)
    if _trace:
        _cache['last_exec_time_ns'] = res.exec_time_ns
    return out
